# revision 2
# baseline (speedup 1.0000x reference)
"""FP8 GEMM kernel for Trainium2 (8 NeuronCores, SPMD data-parallel over tokens).

Computes: out = fp16( fp32( e5m2(x) @ e4m3(weight.T) ) + bias )
  x      [4, 4096, 4096] fp16
  weight [4096, 4096]    fp16  (out_features, in_features)
  bias   [4096]          fp16
  out    [4, 4096, 4096] fp16

Sharding: token dim (B*S = 16384) split across 8 cores (2048 rows each);
weight + bias replicated. No collectives; host concatenates the outputs.

Per-core kernel: DMA-transpose x/w slices into [K-partition] layout (fp16),
cast to fp8 on DVE, then DoubleRow fp8 matmuls (K=256 per instruction)
accumulating in PSUM fp32; bias add fused into the PSUM->SBUF eviction.
"""

import sys

if "/opt/trn_rl_repo" not in sys.path:
    sys.path.insert(0, "/opt/trn_rl_repo")

import numpy as np

B, S, DIN, DOUT = 4, 4096, 4096, 4096
NCORES = 8
M_TOTAL = B * S              # 16384
M_LOC = M_TOTAL // NCORES    # 2048
P = 128
M_TILES = M_LOC // P         # 16
N_TILE = 512
N_TILES = DOUT // N_TILE     # 8
K_SUB = DIN // P             # 32 k-subtiles of 128
K_CHUNKS = K_SUB // 2        # 16 DoubleRow chunks of 256
NCHUNK = 128                 # weight-load granularity along out_features
W_CHUNKS_PER_NTILE = N_TILE // NCHUNK  # 4

_cached_nc = None


def _build():
    global _cached_nc
    if _cached_nc is not None:
        return _cached_nc

    import concourse.mybir as mybir
    import concourse.tile as tile
    from concourse import bacc

    nc = bacc.Bacc("TRN2", target_bir_lowering=False, debug=False,
                   num_devices=NCORES)

    xs = nc.dram_tensor("xs", [M_LOC, DIN], mybir.dt.float16,
                        kind="ExternalInput")
    w = nc.dram_tensor("w", [DOUT, DIN], mybir.dt.float16,
                       kind="ExternalInput")
    bvec = nc.dram_tensor("bvec", [DOUT], mybir.dt.float16,
                          kind="ExternalInput")
    out = nc.dram_tensor("out", [M_LOC, DOUT], mybir.dt.float16,
                         kind="ExternalOutput")

    with tile.TileContext(nc) as tc:
        with tc.tile_pool(name="w8p", bufs=1) as w8p, \
             tc.tile_pool(name="stage", bufs=3) as stagep, \
             tc.tile_pool(name="x8p", bufs=3) as x8p, \
             tc.tile_pool(name="outp", bufs=4) as outp, \
             tc.tile_pool(name="cst", bufs=1) as cst, \
             tc.tile_pool(name="psum", bufs=4, space="PSUM") as psump:

            # bias replicated across the 128 partitions
            bias_rep = cst.tile([P, DOUT], mybir.dt.float16)
            nc.sync.dma_start(bias_rep[:],
                              bvec.ap()[None, :].to_broadcast((P, DOUT)))

            # resident fp8 weight, one tile per n-tile: [ki, ko, n]
            w8 = [w8p.tile([P, K_SUB, N_TILE], mybir.dt.float8e4,
                           tag=f"w8_{j}", name=f"w8_{j}")
                  for j in range(N_TILES)]

            def load_w(j):
                for c in range(W_CHUNKS_PER_NTILE):
                    n0 = j * N_TILE + c * NCHUNK
                    st = stagep.tile([P, K_SUB, NCHUNK], mybir.dt.float16,
                                     tag="stage")
                    nc.sync.dma_start(st[:], w[n0:n0 + NCHUNK, :],
                                      transpose=True)
                    nc.vector.tensor_copy(
                        w8[j][:, :, c * NCHUNK:(c + 1) * NCHUNK], st[:])

            def load_x(m):
                st = stagep.tile([P, K_SUB, P], mybir.dt.float16, tag="stage")
                nc.sync.dma_start(st[:], xs[m * P:(m + 1) * P, :],
                                  transpose=True)
                xt = x8p.tile([P, K_SUB, P], mybir.dt.float8e5, tag="x8")
                nc.vector.tensor_copy(xt[:], st[:])
                return xt

            x_tiles = {0: load_x(0), 1: load_x(1)}
            for j in range(N_TILES):
                load_w(j)

            for m in range(M_TILES):
                if m + 2 < M_TILES:
                    x_tiles[m + 2] = load_x(m + 2)
                xt = x_tiles.pop(m)
                for j in range(N_TILES):
                    ps = psump.tile([P, N_TILE], mybir.dt.float32, tag="ps")
                    for kc in range(K_CHUNKS):
                        nc.tensor.matmul(
                            ps[:],
                            xt[:, 2 * kc:2 * kc + 2, :],
                            w8[j][:, 2 * kc:2 * kc + 2, :],
                            start=(kc == 0),
                            stop=(kc == K_CHUNKS - 1),
                            perf_mode=mybir.MatmulPerfMode.DoubleRow,
                        )
                    ob = outp.tile([P, N_TILE], mybir.dt.float16, tag="ob")
                    nc.vector.tensor_add(
                        ob[:], ps[:],
                        bias_rep[:, j * N_TILE:(j + 1) * N_TILE])
                    nc.sync.dma_start(
                        out[m * P:(m + 1) * P,
                            j * N_TILE:(j + 1) * N_TILE], ob[:])

    nc.compile()
    _cached_nc = nc
    return nc


def kernel(x, weight, bias):
    from concourse.bass_utils import run_bass_kernel_spmd

    x = np.asarray(x)
    weight = np.ascontiguousarray(np.asarray(weight))
    bias = np.ascontiguousarray(np.asarray(bias))
    assert x.dtype == np.float16 and weight.dtype == np.float16

    nc = _build()
    xf = np.ascontiguousarray(x.reshape(M_TOTAL, DIN))
    in_maps = [
        {"xs": xf[c * M_LOC:(c + 1) * M_LOC], "w": weight, "bvec": bias}
        for c in range(NCORES)
    ]
    res = run_bass_kernel_spmd(nc, in_maps, core_ids=list(range(NCORES)))
    out = np.concatenate([r["out"] for r in res.results], axis=0)
    return out.reshape(B, S, DOUT)


# revision 11
# speedup vs baseline: 1.2163x; 1.2163x over previous
"""FP8 GEMM kernel for Trainium2 (8 NeuronCores, SPMD data-parallel over tokens).

Computes: out = fp16( fp32( e5m2(x) @ e4m3(weight.T) ) + bias )
  x      [4, 4096, 4096] fp16
  weight [4096, 4096]    fp16  (out_features, in_features)
  bias   [4096]          fp16
  out    [4, 4096, 4096] fp16

Sharding: token dim (B*S = 16384) split across 8 cores (2048 rows each);
weight + bias replicated. No collectives; host concatenates the outputs.

Layout: the host hands each core K-major views (x_chunk.T [DIN, M_LOC] and
weight.T [DIN, DOUT]) so the contraction dim lands on SBUF partitions with
plain strided DMAs — no XBAR transposes (concurrent XBAR use across two
HWDGE queues corrupts data, and a single queue caps at ~190 GB/s).

Per-core kernel:
 - x loads on the sync HWDGE queue, weight loads on the scalar HWDGE queue
   (independent, both near HBM rate); output stores + bias on SWDGE.
 - fp16 -> fp8 casts on DVE (e5m2 for x, e4m3 for w), matching jax RNE.
 - DoubleRow fp8 matmuls (K=256/instr, moving free dim 2x512) accumulate
   fp32 into PSUM; n-tile outer / m-tile inner loop keeps all of x8
   resident (64KB/part) while w8 n-tiles stream through a 3-deep pool, so
   the PE starts after ~6MB of DMA instead of the whole 33MB weight.
 - Bias add fused into the PSUM eviction on DVE.
"""

import sys

if "/opt/trn_rl_repo" not in sys.path:
    sys.path.insert(0, "/opt/trn_rl_repo")

import numpy as np

B, S, DIN, DOUT = 4, 4096, 4096, 4096
NCORES = 8
M_TOTAL = B * S              # 16384
M_LOC = M_TOTAL // NCORES    # 2048
P = 128
M_TILES = M_LOC // P         # 16 matmul m-tiles of 128 rows
M_PAIRS = M_TILES // 2       # 8 x8 tiles of 256 rows
N_TILE = 512
N_TILES = DOUT // N_TILE     # 8
K_SUB = DIN // P             # 32 k-subtiles of 128
K_CHUNKS = K_SUB // 2        # 16 DoubleRow chunks of 256
WCHUNK = 256                 # weight load granularity (2MB chunks)
W_CHUNKS_PER_NTILE = N_TILE // WCHUNK  # 2

_cached_nc = None


def _build():
    global _cached_nc
    if _cached_nc is not None:
        return _cached_nc

    import concourse.mybir as mybir
    import concourse.tile as tile
    from concourse import bacc

    nc = bacc.Bacc("TRN2", target_bir_lowering=False, debug=False,
                   num_devices=NCORES)

    # K-major inputs (pre-transposed on host)
    xt = nc.dram_tensor("xt", [DIN, M_LOC], mybir.dt.float16,
                        kind="ExternalInput")
    wt = nc.dram_tensor("wt", [DIN, DOUT], mybir.dt.float16,
                        kind="ExternalInput")
    bvec = nc.dram_tensor("bvec", [DOUT], mybir.dt.float16,
                          kind="ExternalInput")
    out = nc.dram_tensor("out", [M_LOC, DOUT], mybir.dt.float16,
                         kind="ExternalOutput")

    # [ki, ko, f] views: k = ko*128 + ki on partitions
    xt_v = xt.ap().rearrange("(ko ki) m -> ki ko m", ki=P)
    wt_v = wt.ap().rearrange("(ko ki) n -> ki ko n", ki=P)

    with tile.TileContext(nc) as tc:
        with tc.tile_pool(name="w8p", bufs=3) as w8p, \
             tc.tile_pool(name="stage", bufs=4) as stagep, \
             tc.tile_pool(name="x8p", bufs=1) as x8p, \
             tc.tile_pool(name="outp", bufs=4) as outp, \
             tc.tile_pool(name="cst", bufs=1) as cst, \
             tc.tile_pool(name="psum", bufs=4, space="PSUM") as psump:

            # bias replicated across the 128 partitions (SWDGE broadcast)
            bias_rep = cst.tile([P, DOUT], mybir.dt.float16)
            nc.gpsimd.dma_start(bias_rep[:],
                                bvec.ap()[None, :].to_broadcast((P, DOUT)))

            # resident fp8 x: 8 tiles of [ki, ko, 256] e5m2
            x8 = [x8p.tile([P, K_SUB, 2 * P], mybir.dt.float8e5,
                           tag=f"x8_{p}", name=f"x8_{p}")
                  for p in range(M_PAIRS)]

            w8 = {}

            def load_w_tile(j):
                w8[j] = w8p.tile([P, K_SUB, N_TILE], mybir.dt.float8e4,
                                 tag="w8", name=f"w8_{j}")

            def load_w_chunk(j, c):
                n0 = j * N_TILE + c * WCHUNK
                st = stagep.tile([P, K_SUB, WCHUNK], mybir.dt.float16,
                                 tag="stage", name=f"wst_{j}_{c}")
                nc.scalar.dma_start(st[:], wt_v[:, :, n0:n0 + WCHUNK])
                nc.vector.tensor_copy(
                    w8[j][:, :, c * WCHUNK:(c + 1) * WCHUNK], st[:])

            def load_x_pair(p):
                st = stagep.tile([P, K_SUB, 2 * P], mybir.dt.float16,
                                 tag="stage", name=f"xst_{p}")
                nc.sync.dma_start(st[:], xt_v[:, :, p * 2 * P:(p + 1) * 2 * P])
                nc.vector.tensor_copy(x8[p][:], st[:])

            # ---- load emission order (per-queue FIFO follows this) ----
            load_w_tile(0)
            for c in range(W_CHUNKS_PER_NTILE):
                load_w_chunk(0, c)
            for p in range(M_PAIRS):
                load_x_pair(p)
            for j in range(1, N_TILES):
                load_w_tile(j)
                for c in range(W_CHUNKS_PER_NTILE):
                    load_w_chunk(j, c)

            # ---- matmul loop: n-tile outer, m-tile inner ----
            for j in range(N_TILES):
                wtile = w8[j]
                for m in range(M_TILES):
                    p, h = divmod(m, 2)
                    ps = psump.tile([P, N_TILE], mybir.dt.float32, tag="ps",
                                    name=f"ps_{j}_{m}")
                    for kc in range(K_CHUNKS):
                        nc.tensor.matmul(
                            ps[:],
                            x8[p][:, 2 * kc:2 * kc + 2, h * P:(h + 1) * P],
                            wtile[:, 2 * kc:2 * kc + 2, :],
                            start=(kc == 0),
                            stop=(kc == K_CHUNKS - 1),
                            perf_mode=mybir.MatmulPerfMode.DoubleRow,
                        )
                    ob = outp.tile([P, N_TILE], mybir.dt.float16, tag="ob",
                                   name=f"ob_{j}_{m}")
                    nc.vector.tensor_add(
                        ob[:], ps[:],
                        bias_rep[:, j * N_TILE:(j + 1) * N_TILE])
                    nc.gpsimd.dma_start(
                        out[m * P:(m + 1) * P,
                            j * N_TILE:(j + 1) * N_TILE], ob[:])

    nc.compile()
    _cached_nc = nc
    return nc


def make_in_maps(x, weight, bias):
    x = np.asarray(x)
    weight = np.asarray(weight)
    bias = np.ascontiguousarray(np.asarray(bias))
    assert x.dtype == np.float16 and weight.dtype == np.float16

    xf = x.reshape(M_TOTAL, DIN)
    wt = np.ascontiguousarray(weight.T)  # [DIN, DOUT]
    return [
        {"xt": np.ascontiguousarray(xf[c * M_LOC:(c + 1) * M_LOC].T),
         "wt": wt, "bvec": bias}
        for c in range(NCORES)
    ]


def gather_out(results):
    out = np.concatenate([r["out"] for r in results], axis=0)
    return out.reshape(B, S, DOUT)


def kernel(x, weight, bias):
    from concourse.bass_utils import run_bass_kernel_spmd

    nc = _build()
    in_maps = make_in_maps(x, weight, bias)
    res = run_bass_kernel_spmd(nc, in_maps, core_ids=list(range(NCORES)))
    return gather_out(res.results)
